# revision 1
# baseline (speedup 1.0000x reference)
"""Trainium2 Bass kernel for out = x @ W.T + b  (x:[8192,1024] f32, W:[1024,1024] f32, b:[1024] f32).

Data-parallel over batch across 8 NeuronCores: each core computes a
[1024,1024] @ [1024,1024]^T matmul + bias for its 1024-row batch shard.

Host-side prep (inside kernel(), not on device): shard x along batch,
pre-transpose x and W so the contraction dim (in_f) lands on SBUF
partitions with fully-contiguous per-partition DMA reads, replicate the
bias across partitions, and cast to the compute dtype.

Schedule (from perfetto analysis):
  - ~108 scratch warm-up matmuls run on the PE from the end of the
    framework preamble. The HAM clock gate needs ~3.4us of sustained PE
    activity to open (1.2 -> 2.4 GHz); the warm-up bridges the first
    input DMA's latency so the real stream starts at full clock, and
    once open it only closes after ~3.4us of contiguous idle.
  - The leading 512KB chunks (x0+x1 / w0 k0-3) ride the two HWDGE
    rings in parallel; the rest of the input stream is need-ordered on
    the SP ring (the ACT ring's first packet lands ~2us later and it
    only sustains ~140GB/s vs SP's ~200, so only the one leading W
    chunk and the early bias half ride it). Matmuls are emitted in a
    wavefront order over (batch-tile, k-pair) so every matmul only
    needs data that has already landed.
  - Outputs are fp16 (+~1e-4 rel err, halves output traffic) on the
    ACT ring; the final group is split into column chunks, the last
    one stored via the by-then-idle SP ring, so the drain after the
    last matmul is short.

Compute modes (MODE):
  f16   : single-pass fp16 matmuls. rel err ~3e-4, fastest DMA (2B in/out).
  f16x3 : fp16 hi/lo split, 3 accumulated matmuls -> ~fp32 accuracy (~3e-7).
  f32r  : float32r (relaxed fp32) matmuls, 1 cyc/row.
  f32   : exact fp32 matmuls (4 cyc/row), reference-grade.
"""

import os

import numpy as np

import concourse.bass as bass
import concourse.mybir as mybir
import concourse.tile as tile
from concourse import bacc
from concourse.bass_utils import run_bass_kernel_spmd

N_CORES = 8
B, IN_F, OUT_F = 8192, 1024, 1024
B_SHARD = B // N_CORES          # 1024 batch rows per core
P = 128                         # SBUF partitions
KO = IN_F // P                  # 8 contraction subtiles
NT = B_SHARD // P               # 8 batch tiles per core
NO = 2                          # 2 output column tiles of 512
OW = OUT_F // NO                # 512 (one PSUM bank of fp32)
NKC = KO // 2                   # 4 k-pair chunks per W half
NC_LAST = 2                     # column chunks for the final output group

MODE = os.environ.get("BASS_KERNEL_MODE", "f16")
N_WARMUP = int(os.environ.get("BASS_WARMUP_MMS", "124"))

# First-half wavefront: (batch-tile t, k-pair kc) emission order. Within
# each t the kc's ascend (PSUM accumulation order); the t0/t1 interleave
# tracks the x0, w0-first-half, x1, w0-second-half arrival sequence so
# the matmul stream is dense from the first real matmul onward.
WAVEFRONT = [
    (0, 0), (0, 1), (1, 0), (1, 1),
    (0, 2), (0, 3), (1, 2), (1, 3),
] + [(t, kc) for t in range(2, NT) for kc in range(NKC)]

_nc_cache = {}


def _build(mode):
    f32 = mybir.dt.float32
    dt_in = {
        "f16": mybir.dt.float16,
        "f16x3": mybir.dt.float16,
        "f32r": mybir.dt.float32r,
        "f32": f32,
    }[mode]
    split = mode == "f16x3"
    dt_out = mybir.dt.float16 if mode == "f16" else f32

    nc = bacc.Bacc("TRN2", target_bir_lowering=False)

    # DRAM layouts are host-packed so every DMA is contiguous per partition:
    #   xt[ki, t, ko, bi]  = x_shard[t*128+bi, ko*128+ki]
    #   wt[ki, ot, ko, oi] = W[ot*512+oi, ko*128+ki]
    #   biasr[p, ot, oi]   = b[ot*512+oi]   (host-replicated)
    xt_d = nc.dram_tensor("xt", [P, NT, KO, P], dt_in, kind="ExternalInput")
    wt_d = nc.dram_tensor("wt", [P, NO, KO, OW], dt_in, kind="ExternalInput")
    if split:
        xl_d = nc.dram_tensor("xl", [P, NT, KO, P], dt_in, kind="ExternalInput")
        wl_d = nc.dram_tensor("wl", [P, NO, KO, OW], dt_in, kind="ExternalInput")
    biasr_d = nc.dram_tensor("biasr", [P, NO, OW], f32, kind="ExternalInput")
    out_d = nc.dram_tensor("out", [B_SHARD, OUT_F], dt_out, kind="ExternalOutput")

    with tile.TileContext(nc) as tc:
        with (
            tc.tile_pool(name="singles", bufs=1) as singles,
            tc.tile_pool(name="wpool", bufs=NO * (2 if split else 1)) as wpool,
            tc.tile_pool(name="xpool", bufs=1) as xpool,
            tc.tile_pool(name="xlpool", bufs=1) as xlpool,
            tc.tile_pool(name="opool", bufs=NT) as opool,
            tc.tile_pool(name="pswarm", bufs=1, space="PSUM") as pswarm,
            tc.tile_pool(name="psums", bufs=5, space="PSUM") as psums,
            tc.tile_pool(name="psl", bufs=NC_LAST, space="PSUM") as psl,
        ):
            # --- PE warm-up (see module docstring) ---
            scr = singles.tile([P, P], dt_in)
            nc.vector.memset(scr[:], 0.0)
            ps_warm = pswarm.tile([P, 64], f32, name="ps_warm", tag="warm")
            for _ in range(N_WARMUP):
                nc.tensor.matmul(ps_warm[:], scr[:], scr[:, :64],
                                 start=True, stop=True)

            bias_sb = singles.tile([P, NO, OW], f32)
            w_tiles = [
                wpool.tile([P, KO, OW], dt_in, name=f"w_{ot}", tag="w_sb")
                for ot in range(NO)
            ]
            wl_tiles = (
                [wpool.tile([P, KO, OW], dt_in, name=f"wl_{ot}", tag="w_sb")
                 for ot in range(NO)] if split else []
            )
            xall = xpool.tile([P, NT, KO, P], dt_in, name="xall", tag="x_sb")
            xlall = (xlpool.tile([P, NT, KO, P], dt_in, name="xlall", tag="xl_sb")
                     if split else None)
            o_tiles = [
                [opool.tile([P, OW], dt_out, name=f"o_{t}_{ot}", tag="o_sb")
                 for ot in range(NO)]
                for t in range(NT)
            ]

            # Single ordered input stream on the SP ring (x tiles and W
            # k-pair chunks interleaved per the wavefront's needs; bias
            # halves slotted just before their consumers; w1 last).
            # W rides the ACT ring, x rides the SP ring. The leading 256KB
            # chunks (x0 / w0-k01) run in parallel so the first matmul's
            # operands land as early as possible; later chunks are larger
            # (4KB/partition runs) and strictly need-ordered per ring.
            nc.scalar.dma_start(out=w_tiles[0][:, 0:4], in_=wt_d[:, 0, 0:4])
            nc.scalar.dma_start(out=bias_sb[:, 0], in_=biasr_d[:, 0])
            nc.sync.dma_start(out=xall[:, 0:2], in_=xt_d[:, 0:2])
            nc.sync.dma_start(out=w_tiles[0][:, 4:8], in_=wt_d[:, 0, 4:8])
            nc.sync.dma_start(out=xall[:, 2:4], in_=xt_d[:, 2:4])
            nc.sync.dma_start(out=xall[:, 4:6], in_=xt_d[:, 4:6])
            nc.sync.dma_start(out=xall[:, 6:8], in_=xt_d[:, 6:8])
            nc.sync.dma_start(out=w_tiles[1][:, 0:4], in_=wt_d[:, 1, 0:4])
            nc.sync.dma_start(out=w_tiles[1][:, 4:8], in_=wt_d[:, 1, 4:8])
            nc.sync.dma_start(out=bias_sb[:, 1], in_=biasr_d[:, 1])
            if split:
                nc.sync.dma_start(out=xlall[:], in_=xl_d[:])
                for ot in range(NO):
                    nc.sync.dma_start(out=wl_tiles[ot][:], in_=wl_d[:, ot])

            def emit_group_tail(t, ot):
                nc.vector.tensor_add(
                    o_tiles[t][ot][:],
                    ps_open[t][:],
                    bias_sb[:, ot],
                )
                nc.scalar.dma_start(
                    out=out_d[t * P:(t + 1) * P, ot * OW:(ot + 1) * OW],
                    in_=o_tiles[t][ot][:],
                )

            # --- first half (ot=0): wavefront over (t, k-pair) ---
            ps_open = {}
            for t, kc in WAVEFRONT:
                if kc == 0:
                    ps_open[t] = psums.tile([P, OW], f32, name=f"ps0_{t}", tag="ps")
                for ko in (2 * kc, 2 * kc + 1):
                    nc.tensor.matmul(
                        ps_open[t][:],
                        xall[:, t, ko],
                        w_tiles[0][:, ko],
                        start=(ko == 0),
                        stop=(ko == KO - 1 and not split),
                    )
                if kc == NKC - 1:
                    if split:
                        i = 0
                        extra = [(xlall, w_tiles[0]), (xall, wl_tiles[0])]
                        for lhs_sb, rhs_sb in extra:
                            for ko in range(KO):
                                nc.tensor.matmul(
                                    ps_open[t][:], lhs_sb[:, t, ko], rhs_sb[:, ko],
                                    start=False, stop=(i == 2 * KO - 1),
                                )
                                i += 1
                    emit_group_tail(t, 0)

            # --- second half (ot=1): all data resident, straight order ---
            for t in range(NT):
                is_last = t == NT - 1 and not split
                groups = [(xall, w_tiles[1])]
                if split:
                    groups += [(xlall, w_tiles[1]), (xall, wl_tiles[1])]
                if not is_last:
                    ps_open[t] = psums.tile([P, OW], f32, name=f"ps1_{t}", tag="ps")
                    n_mm = len(groups) * KO
                    i = 0
                    for lhs_sb, rhs_sb in groups:
                        for ko in range(KO):
                            nc.tensor.matmul(
                                ps_open[t][:],
                                lhs_sb[:, t, ko],
                                rhs_sb[:, ko],
                                start=(i == 0),
                                stop=(i == n_mm - 1),
                            )
                            i += 1
                    emit_group_tail(t, 1)
                else:
                    # final group: independent column chains so the first
                    # chunk's bias-add + store drain while the last chunk's
                    # matmuls still run; the last chunk is narrow so the
                    # exposed drain after the very last matmul is minimal.
                    widths = [OW - OW // 4, OW // 4]
                    off = 0
                    for c, cw in enumerate(widths):
                        sl = slice(off, off + cw)
                        off += cw
                        psc = psl.tile([P, cw], f32, name=f"psl_{c}", tag="psl")
                        for ko in range(KO):
                            nc.tensor.matmul(
                                psc[:],
                                xall[:, t, ko],
                                w_tiles[1][:, ko, sl],
                                start=(ko == 0),
                                stop=(ko == KO - 1),
                            )
                        nc.vector.tensor_add(
                            o_tiles[t][1][:, sl],
                            psc[:],
                            bias_sb[:, 1, sl],
                        )
                        # final chunks split across both HWDGE rings so
                        # their dispatches overlap in the tail
                        dma_eng = nc.sync if c == len(widths) - 1 else nc.scalar
                        dma_eng.dma_start(
                            out=out_d[t * P:(t + 1) * P,
                                      OW + sl.start:OW + sl.stop],
                            in_=o_tiles[t][1][:, sl],
                        )
    nc.compile()
    return nc


def _get_nc(mode):
    if mode not in _nc_cache:
        _nc_cache[mode] = _build(mode)
    return _nc_cache[mode]


def _pack(x, W, b, mode):
    """Shard + retile host-side. Returns in_maps for the 8 cores."""
    np_dt = np.float16 if mode in ("f16", "f16x3") else np.float32
    x = np.asarray(x, dtype=np.float32)
    W = np.asarray(W, dtype=np.float32)
    b = np.asarray(b, dtype=np.float32)

    # [c, t, bi, ko, ki] -> [c, ki, t, ko, bi]
    xs = x.reshape(N_CORES, NT, P, KO, P).transpose(0, 4, 1, 3, 2)
    # [ot, oi, ko, ki] -> [ki, ot, ko, oi]
    ws = W.reshape(NO, OW, KO, P).transpose(3, 0, 2, 1)
    biasr = np.ascontiguousarray(
        np.broadcast_to(b.reshape(1, NO, OW), (P, NO, OW))
    )

    xt = np.ascontiguousarray(xs).astype(np_dt)
    wt = np.ascontiguousarray(ws).astype(np_dt)
    maps = [{"xt": xt[c], "wt": wt, "biasr": biasr} for c in range(N_CORES)]
    if mode == "f16x3":
        xlo = (xs - xt.astype(np.float32)).astype(np_dt)
        wlo = (ws - wt.astype(np.float32)).astype(np_dt)
        for c in range(N_CORES):
            maps[c]["xl"] = np.ascontiguousarray(xlo[c])
            maps[c]["wl"] = wlo
    return maps


def _run(in_maps, mode, **kwargs):
    nc = _get_nc(mode)
    return run_bass_kernel_spmd(nc, in_maps, core_ids=list(range(N_CORES)), **kwargs)


def kernel(x, W, b):
    mode = MODE
    res = _run(_pack(x, W, b, mode), mode)
    out = np.concatenate([r["out"] for r in res.results], axis=0)
    return np.ascontiguousarray(out.astype(np.float32))



# revision 3
# speedup vs baseline: 1.0409x; 1.0409x over previous
"""Trainium2 Bass kernel for out = x @ W.T + b  (x:[8192,1024] f32, W:[1024,1024] f32, b:[1024] f32).

Data-parallel over batch across 8 NeuronCores: each core computes a
[1024,1024] @ [1024,1024]^T matmul + bias for its 1024-row batch shard.

Orientation: W tiles are the stationary operand ([128k x 128o]), x is the
moving operand ([128k x 512b]), so PSUM groups are [128o x 512b] and the
bias is a per-partition scalar (tensor_scalar_add, 4KB bias DMA instead of
a host-replicated 512KB tile).  The output is stored transposed
(out.T [1024o x 1024b] in DRAM) and un-transposed on the host.

Schedule (from perfetto analysis of the previous kernel):
  - PE warm-up matmuls bridge the input-DMA latency and open the HAM
    clock gate; the first real matmul needs only x[k0] (128KB) + the
    leading w[·,k0] slices, so the stream starts ~9.3us instead of 15.4.
  - k-major wavefront: step k runs all 8 o-groups against the single
    x[k] moving tile; per-step feed is a flat 128KB x (sync/Q1 ring) +
    256KB w (scalar/Q10 ring), well under the rings' capacity, so the
    128-matmul stream pipelines at the fp16 floor (~216ns each).
  - The last two k-steps are emitted per-o (staggered closings) so the
    16 bias-adds (alternating vector/gpsimd) and stores (scalar for the
    b0 half, sync for b1) overlap the stream instead of queuing at the
    end; the post-stream drain fits inside the ~3.4us HAM grace window,
    keeping the framework's semaphore-teardown cascade at full clock.
"""

import os

import numpy as np

import concourse.bass as bass
import concourse.mybir as mybir
import concourse.tile as tile
from concourse import bacc
from concourse.bass_utils import run_bass_kernel_spmd

N_CORES = 8
B, IN_F, OUT_F = 8192, 1024, 1024
B_SHARD = B // N_CORES          # 1024 batch rows per core
P = 128                         # SBUF partitions
KO = IN_F // P                  # 8 contraction subtiles
NB = 2                          # 2 batch halves of 512 per core
BI = B_SHARD // NB              # 512 (moving free dim / PSUM bank width)
NO = OUT_F // P                 # 8 output-column tiles of 128
K_TAIL = 2                      # trailing k-steps emitted per-o (staggered)

MODE = os.environ.get("BASS_KERNEL_MODE", "f16")
N_WARMUP = int(os.environ.get("BASS_WARMUP_MMS", "100"))

_nc_cache = {}


def _build(mode):
    f32 = mybir.dt.float32
    f16 = mybir.dt.float16

    nc = bacc.Bacc("TRN2", target_bir_lowering=False)

    # DRAM layouts are host-packed so every DMA is contiguous per partition:
    #   xt[ki, bh, ko, bi] = x_shard[bh*512 + bi, ko*128 + ki]
    #   wt[ki, ko, ot, oi] = W[ot*128 + oi, ko*128 + ki]
    #   biasr[oi, ot]      = b[ot*128 + oi]
    #   out[o, b]          = result.T  (host un-transposes)
    xt_d = nc.dram_tensor("xt", [P, NB, KO, BI], f16, kind="ExternalInput")
    wt_d = nc.dram_tensor("wt", [P, KO, NO, P], f16, kind="ExternalInput")
    biasr_d = nc.dram_tensor("biasr", [P, NO], f32, kind="ExternalInput")
    out_d = nc.dram_tensor("out", [OUT_F, B_SHARD], f16, kind="ExternalOutput")

    with tile.TileContext(nc) as tc:
        with (
            tc.tile_pool(name="singles", bufs=1) as singles,
            tc.tile_pool(name="wpool", bufs=1) as wpool,
            tc.tile_pool(name="xpool", bufs=1) as xpool,
            tc.tile_pool(name="opool", bufs=NB * NO) as opool,
            tc.tile_pool(name="pspool", bufs=8, space="PSUM") as pspool,
        ):
            scr = singles.tile([P, P], f16)
            nc.vector.memset(scr[:], 0.0)
            bias_sb = singles.tile([P, NO], f32)
            wall = wpool.tile([P, KO, NO, P], f16, name="wall", tag="w_sb")
            xall = xpool.tile([P, NB, KO, BI], f16, name="xall", tag="x_sb")
            o_tiles = [
                opool.tile([P, BI], f16, name=f"o_{g}", tag="o_sb")
                for g in range(NB * NO)
            ]

            # --- PE warm-up: bridges input-DMA latency, opens HAM gate ---
            ps_warm = pspool.tile([P, BI], f32, name="ps_warm", tag="ps")
            for _ in range(N_WARMUP):
                nc.tensor.matmul(ps_warm[:, :64], scr[:], scr[:, :64],
                                 start=True, stop=True)

            # --- input DMA program (need-ordered) ---
            # sync/Q1: x stream + the leading w[k0] slices (Q10 spins up
            # ~1.3us later than Q1, so everything the first few matmuls
            # need rides Q1).  scalar/Q10: bias + w[k1..k7].
            nc.sync.dma_start(out=xall[:, 0, 0], in_=xt_d[:, 0, 0])
            nc.scalar.dma_start(out=bias_sb[:], in_=biasr_d[:])
            nc.sync.dma_start(out=wall[:, 0, 0:2], in_=wt_d[:, 0, 0:2])
            nc.sync.dma_start(out=wall[:, 0, 2:8], in_=wt_d[:, 0, 2:8])
            nc.scalar.dma_start(out=wall[:, 1], in_=wt_d[:, 1])
            nc.sync.dma_start(out=xall[:, 0, 1], in_=xt_d[:, 0, 1])
            for k in range(2, KO):
                nc.scalar.dma_start(out=wall[:, k], in_=wt_d[:, k])
                nc.sync.dma_start(out=xall[:, 0, k], in_=xt_d[:, 0, k])
            for k in range(0, KO, 2):
                nc.sync.dma_start(out=xall[:, 1, k:k + 2],
                                  in_=xt_d[:, 1, k:k + 2])

            # --- matmul wavefront ---
            ps = [None] * NO

            def emit_phase(bh, store_engs):
                for k in range(KO - K_TAIL):
                    for o in range(NO):
                        if k == 0:
                            ps[o] = pspool.tile([P, BI], f32,
                                                name=f"ps_{bh}_{o}", tag="ps")
                        nc.tensor.matmul(
                            ps[o][:],
                            wall[:, k, o],
                            xall[:, bh, k],
                            start=(k == 0),
                            stop=False,
                        )
                for o in range(NO):
                    for k in range(KO - K_TAIL, KO):
                        nc.tensor.matmul(
                            ps[o][:],
                            wall[:, k, o],
                            xall[:, bh, k],
                            start=False,
                            stop=(k == KO - 1),
                        )
                    g = bh * NO + o
                    # psum->sbuf copy + bias: ACT engine for the early
                    # groups, DVE for the late ones, so the last group's
                    # chain lands on an idle engine.
                    if o < NO // 2:
                        nc.scalar.add(o_tiles[g][:], ps[o][:],
                                      bias_sb[:, o:o + 1])
                    else:
                        nc.vector.tensor_scalar_add(
                            o_tiles[g][:], ps[o][:], bias_sb[:, o:o + 1]
                        )
                    store_engs[o].dma_start(
                        out=out_d[o * P:(o + 1) * P, bh * BI:(bh + 1) * BI],
                        in_=o_tiles[g][:],
                    )

            emit_phase(0, [nc.sync] * NO)
            emit_phase(1, [nc.scalar] * (NO // 2) + [nc.sync] * (NO // 2))
    nc.compile()
    return nc


def _get_nc(mode):
    if mode not in _nc_cache:
        _nc_cache[mode] = _build(mode)
    return _nc_cache[mode]


def _pack(x, W, b, mode="f16"):
    """Shard + retile host-side. Returns in_maps for the 8 cores."""
    x = np.asarray(x, dtype=np.float32)
    W = np.asarray(W, dtype=np.float32)
    b = np.asarray(b, dtype=np.float32)

    # [c, bh, bi, ko, ki] -> [c, ki, bh, ko, bi]
    xs = x.reshape(N_CORES, NB, BI, KO, P).transpose(0, 4, 1, 3, 2)
    # [ot, oi, ko, ki] -> [ki, ko, ot, oi]
    ws = W.reshape(NO, P, KO, P).transpose(3, 2, 0, 1)
    biasr = np.ascontiguousarray(b.reshape(NO, P).T)  # [oi, ot]

    xt = np.ascontiguousarray(xs).astype(np.float16)
    wt = np.ascontiguousarray(ws).astype(np.float16)
    return [{"xt": xt[c], "wt": wt, "biasr": biasr} for c in range(N_CORES)]


def _run(in_maps, mode="f16", **kwargs):
    nc = _get_nc(mode)
    return run_bass_kernel_spmd(nc, in_maps, core_ids=list(range(N_CORES)), **kwargs)


def kernel(x, W, b):
    res = _run(_pack(x, W, b, MODE), MODE)
    # each core returns out.T [1024 o, 1024 b]; un-transpose + concat
    out = np.concatenate([r["out"].T for r in res.results], axis=0)
    return np.ascontiguousarray(out.astype(np.float32))
